# revision 22
# baseline (speedup 1.0000x reference)
"""Trainium2 Bass kernel for nn_CoAttention.

Data parallel over batch: B=64 split as 8 batches on each of 8 NeuronCores.
Per batch item (Q = x[:384], D = x[384:]):
    QpT = tanh(W @ Q^T + b)              [768, 384]  (PE bf16, ACT tanh+bias)
    L   = Qp @ D^T                       [384, 384]  natural (lhsT=QpT, rhs=D^T)
    E   = exp(L - SHIFT), r2 = rowsum(E)             (ACT exp with accum_out)
    A_D = E * 1/r2 (bf16) = softmax over s
    Rt  = E^T @ [A_D | 1]                [384, 385]  (one matmul gives R AND r1)
    RtN = Rt[:, :384] * 1/r1                          (fused in PSUM->SBUF copy)
    Qp  = QpT^T (18 PE transposes, hidden under step1/L matmul streams)
    Out1 = A_D^T @ Qp                    [384, 768]
    Out2 = RtN^T @ D                     [384, 768]  (== A_D^T @ S1 @ D, no M1)
Outputs are written bf16; the D passthrough third of the output and the
bf16->fp32 upcast are assembled on the host.  All matmul operands are bf16 (same 1 cycle/row PE rate as
float32r, but half the HBM traffic and faster transposes/ldweights).
"""

import numpy as np
import ml_dtypes
from contextlib import ExitStack

N_CORES = 8
BPC = 8          # batches per core
H = 768
T = 384
KT = H // 128    # 6
TT = T // 128    # 3
SHIFT = 60.0

_cache = {}


def _build_nc():
    import concourse.bass as bass
    import concourse.tile as tile
    from concourse import bacc, mybir

    f32 = mybir.dt.float32
    bf16 = mybir.dt.bfloat16
    AF = mybir.ActivationFunctionType

    nc = bacc.Bacc()
    # All params are pre-converted to bf16 on the host -> plain HWDGE
    # copies, no cast passes anywhere, half the HBM traffic.
    xt_h = nc.declare_dram_parameter("xt", [BPC, H, H], bf16, isOutput=False)
    xnb_h = nc.declare_dram_parameter("xnb", [BPC, T, H], bf16, isOutput=False)
    wt_h = nc.declare_dram_parameter("wt", [H, H], bf16, isOutput=False)
    b_h = nc.declare_dram_parameter("bias", [H], f32, isOutput=False)
    eye_h = nc.declare_dram_parameter("eye", [128, 128], bf16, isOutput=False)
    out_h = nc.declare_dram_parameter("out", [BPC, T, 2 * H], bf16, isOutput=True)

    with tile.TileContext(nc) as tc, ExitStack() as ctx:
        consts = ctx.enter_context(tc.tile_pool(name="consts", bufs=1))
        inp0 = ctx.enter_context(tc.tile_pool(name="inp0", bufs=1))
        inp = ctx.enter_context(tc.tile_pool(name="inp", bufs=4))
        qpp = ctx.enter_context(tc.tile_pool(name="qpp", bufs=2))
        ep = ctx.enter_context(tc.tile_pool(name="ep", bufs=2))
        mop = ctx.enter_context(tc.tile_pool(name="mop", bufs=2))
        smallp = ctx.enter_context(tc.tile_pool(name="small", bufs=2))
        pps = ctx.enter_context(tc.tile_pool(name="pps", bufs=5, space="PSUM"))
        ppt = ctx.enter_context(tc.tile_pool(name="ppt", bufs=3, space="PSUM"))

        # Weights in 6 per-ki tiles, alternating HWDGE queues, interleaved
        # with item 0's q^T chunks so the first step1 matmul fires ~2us in.
        wtr = wt_h.rearrange("(ki p) o -> ki p o", p=128)
        xt0 = xt_h[0].rearrange("(ki p) t -> ki p t", p=128)
        wt_k = []
        qt0_k = []
        for ki in range(KT):
            w = consts.tile([128, H], bf16, tag=f"wt{ki}")
            q = inp0.tile([128, T], bf16, tag=f"qt0{ki}")
            eng = nc.sync if ki % 2 == 0 else nc.scalar
            oth = nc.scalar if ki % 2 == 0 else nc.sync
            eng.dma_start(out=w, in_=wtr[ki])
            oth.dma_start(out=q, in_=xt0[ki, :, 0:T])
            wt_k.append(w)
            qt0_k.append(q)
        # small consts on the SWDGE queue, off the critical path
        bias_sb = consts.tile([128, KT], f32)
        nc.sync.dma_start(out=bias_sb, in_=b_h[:].rearrange("(oi p) -> p oi", p=128))
        ident = consts.tile([128, 128], bf16)
        nc.scalar.dma_start(out=ident, in_=eye_h[:, :])
        negshift = consts.tile([128, 1], f32)
        nc.vector.memset(negshift, -SHIFT)

        for b in range(BPC):
            xtb = xt_h[b]
            ob = out_h[b].rearrange("(si p) c -> p si c", p=128)
            last = b == BPC - 1

            if b == 0:
                qts = lambda ki: qt0_k[ki]
            else:
                xqd = inp.tile([128, KT, H], bf16, tag="xqd")
                nc.sync.dma_start(out=xqd,
                                  in_=xtb[:, :].rearrange("(ki p) c -> p ki c", p=128))
                qts = lambda ki, q=xqd: q[:, ki, 0:T]

            # ---- step1: QpT = tanh(W @ Q^T + b) ----
            # The 9 hf0 Qp transposes (reading qpT oi 0-2) hide inside the
            # last two oi groups: their ldweights overlap 162ns L streams.
            qpT = qpp.tile([128, KT, T], bf16, tag="qpT")
            qp = qpp.tile([128, TT, H], bf16, tag="qp")
            t0_ops = [(tti, j) for tti in range(TT) for j in range(TT)]
            tps = None
            ci = 0
            for oi in range(KT):
                psf = pps.tile([128, T + 8], f32, tag="ps")
                ps = psf[:, 0:T]
                for ki in range(KT):
                    nc.tensor.matmul(ps, wt_k[ki][:, oi * 128:(oi + 1) * 128],
                                     qts(ki),
                                     start=(ki == 0), stop=(ki == KT - 1))
                    if oi >= 4 and ci < 9 and (ki % 2 == 0 or oi == KT - 1):
                        tti, j = t0_ops[ci]
                        ci += 1
                        if j == 0:
                            tps = ppt.tile([128, T], bf16, tag="pst")
                        nc.tensor.transpose(tps[:, j * 128:(j + 1) * 128],
                                            qpT[:, j, tti * 128:(tti + 1) * 128],
                                            ident)
                        if j == TT - 1:
                            nc.vector.tensor_copy(qp[:, tti, 0:T], tps)
                nc.scalar.activation(qpT[:, oi, :], ps, AF.Tanh, bias=bias_sb[:, oi:oi + 1])

            # D^T: for item 0 loaded late (after step1) in two half-tiles
            if b == 0:
                dt0a = inp0.tile([128, KT // 2, T], bf16, tag="dt0a")
                nc.sync.dma_start(out=dt0a,
                                  in_=xtb[0:T, T:H].rearrange("(ki p) t -> p ki t", p=128))
                dt0b = inp0.tile([128, KT // 2, T], bf16, tag="dt0b")
                nc.scalar.dma_start(out=dt0b,
                                    in_=xtb[T:H, T:H].rearrange("(ki p) t -> p ki t", p=128))
                dts = lambda ki: (dt0a if ki < KT // 2 else dt0b)[:, ki % (KT // 2), :]
            else:
                dts = lambda ki, q=xqd: q[:, ki, T:H]

            # ---- L natural = Qp @ D^T ; E = exp(L - SHIFT); r2 = rowsum ----
            e_nat = ep.tile([128, TT, T], bf16, tag="e")
            r2 = smallp.tile([128, TT], f32, tag="r2")
            e_sc = ep.tile([128, TT, T + 8], bf16, tag="esc")
            nc.vector.memset(e_sc[:, :, T:T + 1], 1.0)
            for ti in range(TT):
                psf = pps.tile([128, T + 8], f32, tag="ps")
                ps = psf[:, 0:T]
                for ki in range(KT):
                    nc.tensor.matmul(ps, qpT[:, ki, ti * 128:(ti + 1) * 128],
                                     dts(ki),
                                     start=(ki == 0), stop=(ki == KT - 1))
                nc.scalar.activation(e_nat[:, ti, :], ps, AF.Exp, bias=negshift[:, 0:1],
                                     accum_out=r2[:, ti:ti + 1])
                # per-ti normalize: A_D chunk ready ~750ns after its exp
                nc.vector.reciprocal(r2[:, ti:ti + 1], r2[:, ti:ti + 1])
                nc.vector.tensor_scalar_mul(e_sc[:, ti, 0:T], e_nat[:, ti, :],
                                            r2[:, ti:ti + 1])

            # ---- hf1 Qp transposes: fill PE time while ACT exp drains ----
            for tti in range(TT):
                tps = ppt.tile([128, T], bf16, tag="pst")
                for j in range(TT):
                    nc.tensor.transpose(tps[:, j * 128:(j + 1) * 128],
                                        qpT[:, TT + j, tti * 128:(tti + 1) * 128],
                                        ident)
                nc.vector.tensor_copy(qp[:, tti, T:2 * T], tps)

            # D natural (bf16): needed only at Out2
            d_mm = inp.tile([128, TT, H], bf16, tag="dmm")
            nc.scalar.dma_start(out=d_mm, in_=xnb_h[b].rearrange("(n p) h -> p n h", p=128))

            # ---- Rt = E^T @ [A_D | 1] -> R[s',s] and r1[s'] in one matmul ----
            rt = ep.tile([128, TT, T], bf16, tag="rt")
            r1 = smallp.tile([128, TT], f32, tag="r1")
            for si in range(TT):
                psr = pps.tile([128, T + 8], f32, tag="ps")
                for ti in range(TT):
                    nc.tensor.matmul(psr[:, 0:T + 1],
                                     e_nat[:, ti, si * 128:(si + 1) * 128],
                                     e_sc[:, ti, 0:T + 1],
                                     start=(ti == 0), stop=(ti == TT - 1))
                nc.vector.reciprocal(r1[:, si:si + 1], psr[:, T:T + 1])
                nc.vector.tensor_scalar_mul(rt[:, si, :], psr[:, 0:T],
                                            r1[:, si:si + 1])

            # ---- Out1 = A_D^T @ Qp ; Out2 = RtN^T @ D ----
            # copies: o1 on ACT, o2 on DVE; for the last item alternate both
            # across ACT/DVE so the tail drains twice as fast
            o12 = mop.tile([128, TT, 2 * H], bf16, tag="o12")
            cp = 0
            for hf in range(2):
                for si in range(TT):
                    psf = pps.tile([128, T + 8], f32, tag="ps")
                    ps = psf[:, 0:T]
                    for ti in range(TT):
                        nc.tensor.matmul(ps, e_sc[:, ti, si * 128:(si + 1) * 128],
                                         qp[:, ti, hf * T:(hf + 1) * T],
                                         start=(ti == 0), stop=(ti == TT - 1))
                    dst = o12[:, si, hf * T:(hf + 1) * T]
                    if last and cp % 2 == 0:
                        nc.vector.tensor_copy(dst, ps)
                    else:
                        nc.scalar.activation(dst, ps, AF.Copy)
                    cp += 1
                    if last:
                        eng = nc.sync if (si + hf) % 2 == 0 else nc.scalar
                        eng.dma_start(out=ob[:, si:si + 1, hf * T:(hf + 1) * T],
                                      in_=o12[:, si:si + 1, hf * T:(hf + 1) * T])
            cp = 0
            for hf in range(2):
                for si in range(TT):
                    psf = pps.tile([128, T + 8], f32, tag="ps")
                    ps = psf[:, 0:T]
                    for ti in range(TT):
                        nc.tensor.matmul(ps, rt[:, ti, si * 128:(si + 1) * 128],
                                         d_mm[:, ti, hf * T:(hf + 1) * T],
                                         start=(ti == 0), stop=(ti == TT - 1))
                    dst = o12[:, si, H + hf * T:H + (hf + 1) * T]
                    if last and cp % 2 == 0:
                        nc.scalar.activation(dst, ps, AF.Copy)
                    else:
                        nc.vector.tensor_copy(dst, ps)
                    cp += 1
                    if last:
                        eng = nc.sync if (si + hf) % 2 == 0 else nc.scalar
                        eng.dma_start(out=ob[:, si:si + 1, H + hf * T:H + (hf + 1) * T],
                                      in_=o12[:, si:si + 1, H + hf * T:H + (hf + 1) * T])

            if not last:
                eng = nc.sync if b % 2 == 0 else nc.scalar
                eng.dma_start(out=ob, in_=o12)

    nc.compile()
    return nc


def get_nc():
    if "nc" not in _cache:
        _cache["nc"] = _build_nc()
    return _cache["nc"]


def _prep(x, W, b):
    x = np.ascontiguousarray(np.asarray(x, dtype=np.float32))
    WT = np.ascontiguousarray(np.asarray(W, dtype=np.float32).T.astype(ml_dtypes.bfloat16))
    bias = np.ascontiguousarray(np.asarray(b, dtype=np.float32))
    xt = np.ascontiguousarray(np.swapaxes(x, 1, 2).astype(ml_dtypes.bfloat16))
    xnb = np.ascontiguousarray(x[:, T:, :].astype(ml_dtypes.bfloat16))
    eye = np.eye(128, dtype=ml_dtypes.bfloat16)
    in_maps = [{"xt": xt[i * BPC:(i + 1) * BPC], "xnb": xnb[i * BPC:(i + 1) * BPC],
                "wt": WT, "bias": bias, "eye": eye}
               for i in range(N_CORES)]
    return in_maps


def run(x, W, b, trace=False, tmpdir=None):
    from concourse.bass_utils import run_bass_kernel_spmd
    nc = get_nc()
    x = np.asarray(x, dtype=np.float32)
    res = run_bass_kernel_spmd(nc, _prep(x, W, b), list(range(N_CORES)),
                               trace=trace, tmpdir=tmpdir)
    dev = np.concatenate([np.asarray(res.results[i]["out"]) for i in range(N_CORES)],
                         axis=0)
    out = np.empty((BPC * N_CORES, T, 3 * H), np.float32)
    out[:, :, 0:2 * H] = dev.astype(np.float32)
    out[:, :, 2 * H:] = x[:, T:, :]
    return out, res


def kernel(x, W, b):
    return run(x, W, b)[0]
